# revision 1
# baseline (speedup 1.0000x reference)
"""Trainium2 Bass kernel for BatchTemporalContrastiveLoss.

Strategy (pure data-parallel over 8 NeuronCores, batch B=16384 -> 2048/core):
  - Host: cast/transpose inputs to bf16 matmul-friendly layouts, build a
    per-row multiplicity histogram W[b, j] of neg_indices (so the on-device
    "gather" becomes a dense weighted reduction), shard batch across cores.
  - Device (per core), all matmuls in bf16 with fp32 PSUM accumulation:
      L1:  Y1^T = relu(W1^T @ X^T + b1)        (X^T supplied pre-transposed)
      L2:  Y2   = Y1 @ W2 + b2                  (Y1^T tiles are the stationary)
      LN:  with beta==0 (+ g==1, b2==0), LayerNorm + L2-normalize collapse to
           zg = Y2 - mean;  final = zg / ||zg||.  The 1/||zg|| scalar is
           deferred: applied inside the Exp scale (per-partition AP) and the
           pos-similarity product, so the 256-wide vectors are never divided.
           1/sqrt is computed as exp(-0.5*ln(x)) to stay in one ACT table set.
      pos2[b] = 2 * <zg_a, zg_p> / (||zg_a|| ||zg_p||)
      S = zg_a @ negT (PE);  E = exp(S * 2/||zg_a||) (ACT);  negsum = sum_j W*E
      loss_b = ln(exp(pos2) + negsum) - pos2
    Anchor/positive row blocks are interleaved (A0 P0 A1 P1 ...) so the
    negative-similarity and loss work pipelines across the whole kernel.
  - Host: mean of the 16384 per-row losses (float64) -> scalar fp32.
"""

import os
import sys

import numpy as np
import ml_dtypes

_TRN_REPO = "/opt/trn_rl_repo"
if _TRN_REPO not in sys.path:
    sys.path.insert(0, _TRN_REPO)

import concourse.bass as bass
import concourse.bacc as bacc
import concourse.tile as tile
from concourse import mybir
from concourse.bass_utils import run_bass_kernel_spmd

# Steer bacc's activation-table chooser to the one set that contains all of
# relu/copy/square/exp/ln (it greedily picks the first set containing each
# function, causing ~1.3us table swaps between Exp and Ln otherwise). Keep
# positions so act_func_set_id still indexes walrus's act_info.json.
_orig_get_act_tables = bacc.get_activation_tables


def _combined_act_tables(arch):
    tables = _orig_get_act_tables(arch)
    keep = ("natural_log_exp_and_others", "sqrt_and_others")
    return {name: (funcs if name in keep else set())
            for name, funcs in tables.items()}


bacc.get_activation_tables = _combined_act_tables

F32 = mybir.dt.float32
BF16 = mybir.dt.bfloat16
AF = mybir.ActivationFunctionType
ALU = mybir.AluOpType
BF = ml_dtypes.bfloat16

# Problem constants (hardcoded per spec)
B, H, HH, P, K, NBUF = 16384, 2048, 1024, 256, 64, 2000
NCORES = 8
BL = B // NCORES          # 2048 rows per core
JP = 2048                 # negatives padded to 2048 columns
BT = BL // 128            # 16 anchor b-tiles of 128 rows per core

LAST_RESULTS = None       # BassKernelResults of the most recent run
_NC_CACHE = {}


def _emit_fast(tc, out_losses, ins, ident_dram):
    """Interleaved-layout fast path (b2==0, gamma==1, beta==0).

    xt column blocks of 512: [A0 P0 A1 P1 A2 P2 A3 P3]; slab s even = anchor
    group s//2 (tiles 4*(s//2)..+3), s odd = positive group s//2.
    """
    from contextlib import ExitStack

    nc = tc.nc
    with ExitStack() as ctx:
        const = ctx.enter_context(tc.tile_pool(name="const", bufs=1))

        w1s0 = const.tile([128, 1024], BF16, tag="w1s0")
        nc.sync.dma_start(w1s0[:], ins["w1"][0:128, :])
        b1r = const.tile([128, 8], F32, tag="b1r")
        nc.sync.dma_start(b1r[:], ins["b1r"][:, :])
        ident = const.tile([128, 128], BF16, tag="ident")
        ln2c = const.tile([128, 1], F32, tag="ln2c")
        nc.gpsimd.memset(ln2c[:], 0.6931471805599453)

        negt = []
        for c in range(2):
            ngt = const.tile([128, JP], BF16, tag=f"negt{c}")
            negt.append(ngt)

        anchf = [const.tile([128, 256], F32, tag=f"anchf{t}",
                            name=f"anchf{t}") for t in range(BT)]
        anchT = [[const.tile([128, 128], BF16, tag=f"anchT{t}_{c}",
                             name=f"anchT{t}_{c}")
                  for c in range(2)] for t in range(BT)]
        rn2a = [const.tile([128, 1], F32, tag=f"rn2a{t}", name=f"rn2a{t}")
                for t in range(BT)]
        pos2 = [const.tile([128, 1], F32, tag=f"pos2{t}", name=f"pos2{t}")
                for t in range(BT)]
        negsum = [const.tile([128, 1], F32, tag=f"negsum{t}",
                             name=f"negsum{t}") for t in range(BT)]
        lossT = const.tile([128, BT], F32, tag="lossT")

        xpool = ctx.enter_context(tc.tile_pool(name="xk", bufs=32))
        y1pool = ctx.enter_context(tc.tile_pool(name="y1", bufs=16))
        smp = ctx.enter_context(tc.tile_pool(name="small", bufs=6))
        epool = ctx.enter_context(tc.tile_pool(name="ep", bufs=3))
        wpool = ctx.enter_context(tc.tile_pool(name="wp", bufs=3))

        # ---- boot: slab 0 (= A0) with k-outer over 8 open PSUM groups, so
        # PE saturates as soon as the first w1/x chunk pair lands ----
        w1s = [w1s0]
        xk0 = []
        xt_t = xpool.tile([128, 512], BF16, tag="xk", name="xk0_0")
        nc.sync.dma_start(xt_t[:], ins["xt"][0:128, 0:512])
        xk0.append(xt_t)
        for k in range(1, 16):
            w1t = const.tile([128, 1024], BF16, tag=f"w1s{k}",
                             name=f"w1s{k}")
            nc.sync.dma_start(w1t[:], ins["w1"][k * 128:(k + 1) * 128, :])
            w1s.append(w1t)
            xt_t = xpool.tile([128, 512], BF16, tag="xk", name=f"xk0_{k}")
            nc.sync.dma_start(xt_t[:], ins["xt"][k * 128:(k + 1) * 128, 0:512])
            xk0.append(xt_t)
        w2s = []
        for k in range(8):
            w2t = const.tile([128, 256], BF16, tag=f"w2s{k}")
            w2s.append(w2t)

        y1_prev = []
        with tc.tile_pool(name="bootps", bufs=8, space="PSUM") as bootps:
            ps0 = []
            for n1 in range(8):
                bp = bootps.tile([128, 512], F32, tag="boot")
                ps0.append(bp)
            for k in range(16):
                for n1 in range(8):
                    nc.tensor.matmul(
                        ps0[n1][:],
                        w1s[k][:, n1 * 128:(n1 + 1) * 128],
                        xk0[k][:],
                        start=(k == 0),
                        stop=(k == 15),
                    )
            for n1 in range(8):
                y1_t = y1pool.tile([128, 512], BF16, tag="y1")
                nc.scalar.activation(
                    y1_t[:], ps0[n1][:], AF.Relu, bias=b1r[:, n1:n1 + 1],
                    scale=1.0
                )
                y1_prev.append(y1_t)

        l1ps = ctx.enter_context(tc.tile_pool(name="l1ps", bufs=3, space="PSUM"))
        l2ps = ctx.enter_context(tc.tile_pool(name="l2ps", bufs=3, space="PSUM"))
        # transposes share the S-matmul psum slots (same tag, max-sized)
        sps = ctx.enter_context(tc.tile_pool(name="sps", bufs=2, space="PSUM"))

        def emit_l1(s, k_outer=False):
            xk = []
            for k in range(16):
                xt_t = xpool.tile([128, 512], BF16, tag="xk")
                nc.sync.dma_start(
                    xt_t[:],
                    ins["xt"][k * 128:(k + 1) * 128, s * 512:(s + 1) * 512]
                )
                xk.append(xt_t)
            y1 = [None] * 8
            # k-outer consumes each freshly-DMA'd x chunk for several groups
            # at once (keeps PE busy when the slab's DMA is still in flight)
            groups = [(0, 1, 2), (3, 4, 5), (6, 7)] if k_outer \
                else [(n1,) for n1 in range(8)]
            for grp in groups:
                pss = {n1: l1ps.tile([128, 512], F32, tag="l1",
                                     name=f"l1ps_{s}_{n1}") for n1 in grp}
                for k in range(16):
                    for n1 in grp:
                        nc.tensor.matmul(
                            pss[n1][:],
                            w1s[k][:, n1 * 128:(n1 + 1) * 128],
                            xk[k][:],
                            start=(k == 0),
                            stop=(k == 15),
                        )
                for n1 in grp:
                    y1_t = y1pool.tile([128, 512], BF16, tag="y1",
                                       name=f"y1_{s}_{n1}")
                    nc.scalar.activation(
                        y1_t[:], pss[n1][:], AF.Relu, bias=b1r[:, n1:n1 + 1],
                        scale=1.0
                    )
                    y1[n1] = y1_t
            return y1

        def emit_l2(s, y1):
            sp = s // 2
            is_anchor = (s % 2 == 0)
            for bsub in range(4):
                t = sp * 4 + bsub
                ps2 = l2ps.tile([128, 256], F32, tag="l2")
                for k2 in range(8):
                    nc.tensor.matmul(
                        ps2[:],
                        y1[k2][:, bsub * 128:(bsub + 1) * 128],
                        w2s[k2][:],
                        start=(k2 == 0),
                        stop=(k2 == 7),
                    )
                stats = smp.tile([128, 6], F32, tag="stats")
                nc.vector.bn_stats(stats[:], ps2[:])
                aggr = smp.tile([128, 2], F32, tag="aggr")
                nc.vector.bn_aggr(aggr[:], stats[:])
                # anchors write zg straight into their persistent tile
                zg = anchf[t] if is_anchor \
                    else smp.tile([128, 256], F32, tag="zg", name="zg")
                nc.vector.tensor_scalar_sub(zg[:], ps2[:], aggr[:, 0:1])
                nsq = smp.tile([128, 1], F32, tag="nsq")
                dump = smp.tile([128, 256], F32, tag="dump")
                nc.scalar.activation(dump[:], zg[:], AF.Square, bias=0.0,
                                     scale=1.0, accum_out=nsq[:])
                lnq = smp.tile([128, 1], F32, tag="lnq")
                nc.scalar.activation(lnq[:], nsq[:], AF.Ln, bias=0.0, scale=1.0)
                if is_anchor:
                    # rn2a = min(2/||zg||, 2e12) = min(exp(-ln(nsq)/2+ln2),..)
                    r2 = smp.tile([128, 1], F32, tag="r2")
                    nc.scalar.activation(r2[:], lnq[:], AF.Exp,
                                         bias=ln2c[:, 0:1], scale=-0.5)
                    nc.vector.tensor_scalar_min(rn2a[t][:], r2[:], 2e12)
                    abf = smp.tile([128, 256], BF16, tag="abf")
                    nc.scalar.copy(abf[:], zg[:])
                    for c in range(2):
                        pst = sps.tile([128, 128], BF16, tag="sp")
                        nc.tensor.transpose(pst[:],
                                            abf[:, c * 128:(c + 1) * 128],
                                            ident[:])
                        nc.vector.tensor_copy(anchT[t][c][:], pst[:])
                else:
                    rp = smp.tile([128, 1], F32, tag="rp")
                    nc.scalar.activation(rp[:], lnq[:], AF.Exp, bias=0.0,
                                         scale=-0.5)
                    rpm = smp.tile([128, 1], F32, tag="rpm")
                    nc.vector.tensor_scalar_min(rpm[:], rp[:], 1e12)
                    prod = smp.tile([128, 256], F32, tag="prod")
                    nc.vector.tensor_tensor(prod[:], zg[:], anchf[t][:],
                                            ALU.mult)
                    praw = smp.tile([128, 1], F32, tag="praw")
                    nc.vector.reduce_sum(praw[:], prod[:],
                                         axis=mybir.AxisListType.X)
                    # pos2 = 2 * praw / (||zg_a|| ||zg_p||)
                    p1 = smp.tile([128, 1], F32, tag="p1")
                    nc.vector.tensor_tensor(p1[:], praw[:], rn2a[t][:],
                                            ALU.mult)
                    nc.vector.tensor_tensor(pos2[t][:], p1[:], rpm[:],
                                            ALU.mult)

        def emit_neg_tile(t):
            wct = wpool.tile([128, JP], BF16, tag="wc")
            nc.sync.dma_start(wct[:], ins["wcnt"][t * 128:(t + 1) * 128, :])
            E = epool.tile([128, JP], BF16, tag="E")
            for j in range(4):
                ps = sps.tile([128, 512], F32, tag="sp")
                for c in range(2):
                    nc.tensor.matmul(
                        ps[:],
                        anchT[t][c][:],
                        negt[c][:, j * 512:(j + 1) * 512],
                        start=(c == 0),
                        stop=(c == 1),
                    )
                nc.scalar.activation(E[:, j * 512:(j + 1) * 512], ps[:],
                                     AF.Exp, bias=0.0, scale=rn2a[t][:, 0:1])
            prodE = epool.tile([128, JP], BF16, tag="prodE")
            nc.vector.tensor_tensor(prodE[:], E[:], wct[:], ALU.mult)
            nc.vector.reduce_sum(negsum[t][:], prodE[:],
                                 axis=mybir.AxisListType.X)

        def emit_loss(t):
            pe = smp.tile([128, 1], F32, tag="pe")
            nc.scalar.activation(pe[:], pos2[t][:], AF.Exp, bias=0.0, scale=1.0)
            tot = smp.tile([128, 1], F32, tag="tot")
            nc.vector.tensor_tensor(tot[:], pe[:], negsum[t][:], ALU.add)
            lse = smp.tile([128, 1], F32, tag="lse")
            nc.scalar.activation(lse[:], tot[:], AF.Ln, bias=0.0, scale=1.0)
            nc.vector.tensor_tensor(lossT[:, t:t + 1], lse[:], pos2[t][:],
                                    ALU.subtract)

        # ---- pipelined steady state ----
        for s in range(1, 8):
            y1_next = emit_l1(s, k_outer=(s == 1))
            if s == 1:
                # needed only from L2(A0) onwards: don't delay slab-1 x DMA
                nc.sync.dma_start(ident[:], ident_dram[:, :])
                for k in range(8):
                    nc.sync.dma_start(w2s[k][:],
                                      ins["w2"][k * 128:(k + 1) * 128, :])
                for c in range(2):
                    nc.sync.dma_start(negt[c][:],
                                      ins["negt"][c * 128:(c + 1) * 128, :])
            emit_l2(s - 1, y1_prev)
            y1_prev = y1_next
            grp = (s - 1) // 2
            tiles = range(grp * 4, grp * 4 + 4)
            if (s - 1) % 2 == 0:
                for t in tiles:
                    emit_neg_tile(t)
            else:
                for t in tiles:
                    emit_loss(t)
        emit_l2(7, y1_prev)
        for t in range(12, BT):
            emit_loss(t)

        nc.sync.dma_start(out_losses[:, :], lossT[:])


def _emit_general(tc, out_losses, ins, ident_dram):
    """General path (nonzero beta/b2 or gamma != 1): sequential layout
    [A0..A3 P0..P3], full LayerNorm with explicit normalization."""
    from contextlib import ExitStack

    nc = tc.nc
    with ExitStack() as ctx:
        const = ctx.enter_context(tc.tile_pool(name="const", bufs=1))

        b1r = const.tile([128, 8], F32, tag="b1r")
        nc.sync.dma_start(b1r[:], ins["b1r"][:, :])
        b2r = const.tile([128, 256], F32, tag="b2r")
        nc.sync.dma_start(b2r[:], ins["b2r"][:, :])
        gr = const.tile([128, 256], F32, tag="gr")
        nc.sync.dma_start(gr[:], ins["gr"][:, :])
        br = const.tile([128, 256], F32, tag="br")
        nc.sync.dma_start(br[:], ins["br"][:, :])
        ident = const.tile([128, 128], BF16, tag="ident")
        nc.sync.dma_start(ident[:], ident_dram[:, :])

        negt = []
        for c in range(2):
            ngt = const.tile([128, JP], BF16, tag=f"negt{c}")
            negt.append(ngt)

        anchf = const.tile([128, BT * 256], F32, tag="anchf")
        anchT = []
        for c in range(2):
            at = const.tile([128, BL], BF16, tag=f"anchT{c}")
            anchT.append(at)
        pos2 = const.tile([128, BT], F32, tag="pos2")
        lossT = const.tile([128, BT], F32, tag="lossT")
        negsumT = const.tile([128, BT], F32, tag="negsumT")

        xpool = ctx.enter_context(tc.tile_pool(name="xk", bufs=32))
        y1pool = ctx.enter_context(tc.tile_pool(name="y1", bufs=16))
        smp = ctx.enter_context(tc.tile_pool(name="small", bufs=4))
        epool = ctx.enter_context(tc.tile_pool(name="ep", bufs=2))
        wpool = ctx.enter_context(tc.tile_pool(name="wp", bufs=2))

        w1s = []
        for k in range(16):
            w1t = const.tile([128, 1024], BF16, tag=f"w1s{k}")
            nc.sync.dma_start(w1t[:], ins["w1"][k * 128:(k + 1) * 128, :])
            w1s.append(w1t)
        w2s = []
        for k in range(8):
            w2t = const.tile([128, 256], BF16, tag=f"w2s{k}")
            nc.sync.dma_start(w2t[:], ins["w2"][k * 128:(k + 1) * 128, :])
            w2s.append(w2t)
        for c in range(2):
            nc.sync.dma_start(negt[c][:], ins["negt"][c * 128:(c + 1) * 128, :])

        l1ps = ctx.enter_context(tc.tile_pool(name="l1ps", bufs=3, space="PSUM"))
        l2ps = ctx.enter_context(tc.tile_pool(name="l2ps", bufs=2, space="PSUM"))
        sps = ctx.enter_context(tc.tile_pool(name="sps", bufs=2, space="PSUM"))
        tps = ctx.enter_context(tc.tile_pool(name="tps", bufs=1, space="PSUM"))

        def emit_slab(s):
            xk = []
            for k in range(16):
                xt_t = xpool.tile([128, 512], BF16, tag="xk")
                nc.sync.dma_start(
                    xt_t[:],
                    ins["xt"][k * 128:(k + 1) * 128, s * 512:(s + 1) * 512]
                )
                xk.append(xt_t)
            y1 = []
            for n1 in range(8):
                ps = l1ps.tile([128, 512], F32, tag="l1")
                for k in range(16):
                    nc.tensor.matmul(
                        ps[:],
                        w1s[k][:, n1 * 128:(n1 + 1) * 128],
                        xk[k][:],
                        start=(k == 0),
                        stop=(k == 15),
                    )
                y1_t = y1pool.tile([128, 512], BF16, tag="y1")
                nc.scalar.activation(
                    y1_t[:], ps[:], AF.Relu, bias=b1r[:, n1:n1 + 1], scale=1.0
                )
                y1.append(y1_t)

            for bsub in range(4):
                t = s * 4 + bsub
                ps2 = l2ps.tile([128, 256], F32, tag="l2")
                for k2 in range(8):
                    nc.tensor.matmul(
                        ps2[:],
                        y1[k2][:, bsub * 128:(bsub + 1) * 128],
                        w2s[k2][:],
                        start=(k2 == 0),
                        stop=(k2 == 7),
                    )
                y2 = smp.tile([128, 256], F32, tag="y2")
                nc.vector.tensor_tensor(y2[:], ps2[:], b2r[:], ALU.add)
                stats = smp.tile([128, 6], F32, tag="stats")
                nc.vector.bn_stats(stats[:], y2[:])
                aggr = smp.tile([128, 2], F32, tag="aggr")
                nc.vector.bn_aggr(aggr[:], stats[:])
                veps = smp.tile([128, 1], F32, tag="veps")
                nc.vector.tensor_scalar_add(veps[:], aggr[:, 1:2], 1e-5)
                std = smp.tile([128, 1], F32, tag="std")
                nc.scalar.activation(std[:], veps[:], AF.Sqrt, bias=0.0,
                                     scale=1.0)
                rstd = smp.tile([128, 1], F32, tag="rstd")
                nc.vector.reciprocal(rstd[:], std[:])
                xln = smp.tile([128, 256], F32, tag="xln")
                nc.vector.tensor_scalar(
                    xln[:], y2[:], aggr[:, 0:1], rstd[:], ALU.subtract,
                    ALU.mult)
                xg = smp.tile([128, 256], F32, tag="xg")
                nc.vector.tensor_tensor(xg[:], xln[:], gr[:], ALU.mult)
                xb = smp.tile([128, 256], F32, tag="xb")
                nc.vector.tensor_tensor(xb[:], xg[:], br[:], ALU.add)
                nsq = smp.tile([128, 1], F32, tag="nsq")
                dump = smp.tile([128, 256], F32, tag="dump")
                nc.scalar.activation(dump[:], xb[:], AF.Square, bias=0.0,
                                     scale=1.0, accum_out=nsq[:])
                nrm = smp.tile([128, 1], F32, tag="nrm")
                nc.scalar.activation(nrm[:], nsq[:], AF.Sqrt, bias=0.0,
                                     scale=1.0)
                nmx = smp.tile([128, 1], F32, tag="nmx")
                nc.vector.tensor_scalar_max(nmx[:], nrm[:], 1e-12)
                rn = smp.tile([128, 1], F32, tag="rn")
                nc.vector.reciprocal(rn[:], nmx[:])
                if t < BT:
                    nc.vector.tensor_scalar(
                        anchf[:, t * 256:(t + 1) * 256], xb[:], rn[:], None,
                        ALU.mult)
                    abf = smp.tile([128, 256], BF16, tag="abf")
                    nc.scalar.copy(abf[:], anchf[:, t * 256:(t + 1) * 256])
                    for c in range(2):
                        pst = tps.tile([128, 128], BF16, tag="tp")
                        nc.tensor.transpose(pst[:],
                                            abf[:, c * 128:(c + 1) * 128],
                                            ident[:])
                        nc.vector.tensor_copy(
                            anchT[c][:, t * 128:(t + 1) * 128], pst[:])
                else:
                    ta = t - BT
                    posf = smp.tile([128, 256], F32, tag="posf")
                    nc.vector.tensor_scalar(posf[:], xb[:], rn[:], None,
                                            ALU.mult)
                    prod = smp.tile([128, 256], F32, tag="prod")
                    nc.vector.tensor_tensor(
                        prod[:], posf[:], anchf[:, ta * 256:(ta + 1) * 256],
                        ALU.mult)
                    psim = smp.tile([128, 1], F32, tag="psim")
                    nc.vector.reduce_sum(psim[:], prod[:],
                                         axis=mybir.AxisListType.X)
                    nc.vector.tensor_scalar_mul(pos2[:, ta:ta + 1], psim[:],
                                                2.0)

        def emit_neg_tile(t):
            wct = wpool.tile([128, JP], BF16, tag="wc")
            nc.sync.dma_start(wct[:], ins["wcnt"][t * 128:(t + 1) * 128, :])
            E = epool.tile([128, JP], BF16, tag="E")
            for j in range(4):
                ps = sps.tile([128, 512], F32, tag="sp")
                for c in range(2):
                    nc.tensor.matmul(
                        ps[:],
                        anchT[c][:, t * 128:(t + 1) * 128],
                        negt[c][:, j * 512:(j + 1) * 512],
                        start=(c == 0),
                        stop=(c == 1),
                    )
                nc.scalar.activation(E[:, j * 512:(j + 1) * 512], ps[:],
                                     AF.Exp, bias=0.0, scale=2.0)
            prodE = epool.tile([128, JP], BF16, tag="prodE")
            nc.vector.tensor_tensor(prodE[:], E[:], wct[:], ALU.mult)
            nc.vector.reduce_sum(negsumT[:, t:t + 1], prodE[:],
                                 axis=mybir.AxisListType.X)

        for s in range(4):
            emit_slab(s)
        neg_sched = {4: range(0, 4), 5: range(4, 8), 6: range(8, 12),
                     7: range(12, 16)}
        for s in range(4, 8):
            emit_slab(s)
            for t in neg_sched[s]:
                emit_neg_tile(t)

        peT = smp.tile([128, BT], F32, tag="peT")
        nc.scalar.activation(peT[:], pos2[:], AF.Exp, bias=0.0, scale=1.0)
        totT = smp.tile([128, BT], F32, tag="totT")
        nc.vector.tensor_tensor(totT[:], peT[:], negsumT[:], ALU.add)
        lseT = smp.tile([128, BT], F32, tag="lseT")
        nc.scalar.activation(lseT[:], totT[:], AF.Ln, bias=0.0, scale=1.0)
        nc.vector.tensor_tensor(lossT[:], lseT[:], pos2[:], ALU.subtract)

        nc.sync.dma_start(out_losses[:, :], lossT[:])


def build_program(fast=True):
    if fast in _NC_CACHE:
        return _NC_CACHE[fast]
    nc = bacc.Bacc("TRN2", target_bir_lowering=False, debug=False,
                   num_devices=NCORES)
    ins = {
        "xt": nc.dram_tensor("xt", [H, 2 * BL], BF16, kind="ExternalInput").ap(),
        "w1": nc.dram_tensor("w1", [H, HH], BF16, kind="ExternalInput").ap(),
        "w2": nc.dram_tensor("w2", [HH, P], BF16, kind="ExternalInput").ap(),
        "b1r": nc.dram_tensor("b1r", [128, 8], F32, kind="ExternalInput").ap(),
        "b2r": nc.dram_tensor("b2r", [128, 256], F32, kind="ExternalInput").ap(),
        "gr": nc.dram_tensor("gr", [128, 256], F32, kind="ExternalInput").ap(),
        "br": nc.dram_tensor("br", [128, 256], F32, kind="ExternalInput").ap(),
        "negt": nc.dram_tensor("negt", [P, JP], BF16, kind="ExternalInput").ap(),
        "wcnt": nc.dram_tensor("wcnt", [BL, JP], BF16, kind="ExternalInput").ap(),
    }
    out = nc.dram_tensor("losses", [128, BT], F32, kind="ExternalOutput").ap()
    ident_dram = nc.inline_tensor(np.eye(128, dtype=BF), "ident").ap()
    with tile.TileContext(nc) as tc:
        if fast:
            _emit_fast(tc, out, ins, ident_dram)
        else:
            _emit_general(tc, out, ins, ident_dram)
    nc.compile()
    _NC_CACHE[fast] = nc
    return nc


def prepare_in_maps(hidden_states, positive_hidden, neg_buffer, W1, b1, W2, b2,
                    ln_gamma, ln_beta, neg_indices):
    hidden_states = np.asarray(hidden_states, dtype=np.float32)
    positive_hidden = np.asarray(positive_hidden, dtype=np.float32)
    neg_buffer = np.asarray(neg_buffer, dtype=np.float32)
    idx = np.asarray(neg_indices).astype(np.int64)

    g = np.asarray(ln_gamma, dtype=np.float32)
    beta = np.asarray(ln_beta, dtype=np.float32)
    b2a = np.asarray(b2, dtype=np.float32)
    # fast path: LayerNorm rstd cancels against the L2 norm when beta==0 and
    # the remaining affine pieces are identity.
    fast = bool(np.all(beta == 0.0) and np.all(g == 1.0) and np.all(b2a == 0.0))

    w1b = np.ascontiguousarray(np.asarray(W1, dtype=np.float32).astype(BF))
    w2b = np.ascontiguousarray(np.asarray(W2, dtype=np.float32).astype(BF))
    b1r = np.ascontiguousarray(
        np.asarray(b1, dtype=np.float32).reshape(8, 128).T)
    b2r = np.ascontiguousarray(np.tile(b2a, (128, 1)))
    gr = np.ascontiguousarray(np.tile(g, (128, 1)))
    br = np.ascontiguousarray(np.tile(beta, (128, 1)))
    negt = np.zeros((P, JP), BF)
    negt[:, :NBUF] = neg_buffer.astype(BF).T

    flat = (np.arange(B, dtype=np.int64)[:, None] * JP + idx).ravel()
    wcnt = np.bincount(flat, minlength=B * JP).reshape(B, JP).astype(np.float32)
    wcnt = wcnt.astype(BF)

    in_maps = []
    for c in range(NCORES):
        rows = slice(c * BL, (c + 1) * BL)
        hs, ps = hidden_states[rows], positive_hidden[rows]
        if fast:
            # interleave 512-row blocks: A0 P0 A1 P1 A2 P2 A3 P3
            blocks = []
            for sp in range(4):
                blocks.append(hs[sp * 512:(sp + 1) * 512])
                blocks.append(ps[sp * 512:(sp + 1) * 512])
            xcat = np.concatenate(blocks, axis=0)
        else:
            xcat = np.concatenate([hs, ps], axis=0)
        xt_c = np.ascontiguousarray(xcat.astype(BF).T)  # [H, 2*BL]
        in_maps.append({
            "xt": xt_c, "w1": w1b, "w2": w2b, "b1r": b1r, "b2r": b2r,
            "gr": gr, "br": br, "negt": negt,
            "wcnt": np.ascontiguousarray(wcnt[rows]),
        })
    return in_maps, fast


def kernel(**inputs) -> np.ndarray:
    global LAST_RESULTS
    in_maps, fast = prepare_in_maps(**inputs)
    nc = build_program(fast)
    trace = bool(os.environ.get("BASS_TRACE_KERNEL"))
    res = run_bass_kernel_spmd(nc, in_maps, core_ids=list(range(NCORES)),
                               trace=trace)
    LAST_RESULTS = res
    total = np.float64(0.0)
    for c in range(NCORES):
        total += np.asarray(res.results[c]["losses"], dtype=np.float64).sum()
    return np.array(total / B, dtype=np.float32)

